# revision 8
# baseline (speedup 1.0000x reference)
"""Multi-head causal attention (B=2, S=2048, D=2048, 16 heads) on 8 TRN2 cores.

Sharding: 2-way batch parallel x 4-way head tensor-parallel (4 heads/core).
Each core computes q/k/v projections for its 4 heads, causal softmax
attention, and a partial o-projection; the host sums the 4 partials per batch.

Host pre-transposes x and the weight slices so every on-chip matmul has its
contraction dim on SBUF partitions (no on-chip transposes at all):
  xT  [D, S]   = x[b].T
  wqT [D, JC]  = wq[j0:j0+512, :].T     (same wkT, wvT)
  woT [JC, D]  = wo[:, j0:j0+512].T

On-chip dataflow (per core), all matmuls in float32r (FP22 multiply,
fp32 accumulate in PSUM):
  phase 1: qT[j,s], kT[j,s] (lhsT=wT tile, rhs=xT tile) and v[s,dv]
           (lhsT=xT tile, rhs=wvT tile), spilled to DRAM scratch.
  phase 2 (per head): scoresT[j,i] = k_h @ q_h.T; exp on ScalarE (scale
           fused); causal handled by skipping j>i tiles + masking diagonal
           tiles; denominator via ones[128,128] @ probsT (gives the column
           sums broadcast across all partitions); attT[dv,i] accumulated
           with v tiles stationary; normalized by DVE reciprocal+mul.
  phase 3: out[s,m] partial = sum_h attT_h.T @ woT_h, DMA'd to DRAM.
"""

import math

import numpy as np

B, S, D = 2, 2048, 2048
HEADS, HEAD_DIM = 16, 128
P = 128
JC = 512          # per-core projection width (4 heads x 128)
SC = 512          # s-chunk / matmul moving width
DT = D // P       # 16 contraction tiles
NSC = S // SC     # 4 s-chunks
NST = S // P      # 16 s-tiles
HPC = 4           # heads per core
N_CORES = 8
SCALE = 1.0 / math.sqrt(HEAD_DIM)

_NC_CACHE = None


def build_module():
    """Build + compile the (single-program SPMD) Bass module once."""
    global _NC_CACHE
    if _NC_CACHE is not None:
        return _NC_CACHE

    from contextlib import ExitStack

    import concourse.tile as tile
    from concourse import bacc
    import concourse.mybir as mybir

    f32r = mybir.dt.float32r
    f32 = mybir.dt.float32
    FT = mybir.ActivationFunctionType

    nc = bacc.Bacc(
        "TRN2", target_bir_lowering=False, debug=False, num_devices=N_CORES
    )

    xT = nc.dram_tensor("xT", [D, S], f32r, kind="ExternalInput").ap()
    wqT = nc.dram_tensor("wqT", [D, JC], f32r, kind="ExternalInput").ap()
    wkT = nc.dram_tensor("wkT", [D, JC], f32r, kind="ExternalInput").ap()
    wvT = nc.dram_tensor("wvT", [D, JC], f32r, kind="ExternalInput").ap()
    woT = nc.dram_tensor("woT", [JC, D], f32r, kind="ExternalInput").ap()
    # mask[p, t*SC + i] : causal mask for the diagonal-chunk tile at offset t
    mask = nc.dram_tensor("mask", [P, 4 * SC], f32r, kind="ExternalInput").ap()
    ones = nc.dram_tensor("ones", [P, P], f32r, kind="ExternalInput").ap()
    out = nc.dram_tensor("out", [S, D], f32, kind="ExternalOutput").ap()

    qTd = nc.dram_tensor("qTd", [JC, S], f32r, kind="Internal").ap()
    kTd = nc.dram_tensor("kTd", [JC, S], f32r, kind="Internal").ap()
    vd = nc.dram_tensor("vd", [S, JC], f32r, kind="Internal").ap()

    with tile.TileContext(nc) as tc, ExitStack() as ctx:
        consts = ctx.enter_context(tc.tile_pool(name="consts", bufs=1))
        stage = ctx.enter_context(tc.tile_pool(name="stage", bufs=4))
        psum = ctx.enter_context(tc.tile_pool(name="psum", bufs=2, space="PSUM"))

        mask_sb = consts.tile([P, 4 * SC], f32r, tag="mask", name="mask_sb")
        nc.sync.dma_start(mask_sb, mask)
        ones_sb = consts.tile([P, P], f32r, tag="ones", name="ones_sb")
        nc.sync.dma_start(ones_sb, ones)

        # ---------- Phase 1: q/k/v projections, spilled to DRAM ----------
        # Scoped pools: weights (96KB/part) + x chunks (64KB/part) release
        # before the attention pools are created (SBUF is a stack allocator).
        with ExitStack() as p1:
            wpool = p1.enter_context(tc.tile_pool(name="wpool", bufs=1))
            xpool = p1.enter_context(tc.tile_pool(name="xpool", bufs=2))

            wq_sb = wpool.tile([P, DT, JC], f32r, tag="wq", name="wq_sb")
            nc.sync.dma_start(wq_sb, wqT.rearrange("(dt p) j -> p dt j", p=P))
            wk_sb = wpool.tile([P, DT, JC], f32r, tag="wk", name="wk_sb")
            nc.sync.dma_start(wk_sb, wkT.rearrange("(dt p) j -> p dt j", p=P))
            wv_sb = wpool.tile([P, DT, JC], f32r, tag="wv", name="wv_sb")
            nc.sync.dma_start(wv_sb, wvT.rearrange("(dt p) j -> p dt j", p=P))

            xT_r = xT.rearrange("(dt p) s -> p dt s", p=P)
            for sc in range(NSC):
                xc = xpool.tile([P, DT, SC], f32r, tag="x", name=f"xc_{sc}")
                nc.sync.dma_start(xc, xT_r[:, :, sc * SC:(sc + 1) * SC])
                for w_sb, outd, transposed in (
                    (wq_sb, qTd, True),
                    (wk_sb, kTd, True),
                    (wv_sb, vd, False),
                ):
                    for t in range(4):
                        ps = psum.tile([P, 512], f32, tag="pj", name="ps_proj")
                        for dt in range(DT):
                            if transposed:
                                # qT/kT tile [j, s] = w_slice.T @ x_chunk
                                nc.tensor.matmul(
                                    ps,
                                    lhsT=w_sb[:, dt, t * P:(t + 1) * P],
                                    rhs=xc[:, dt, :],
                                    start=(dt == 0),
                                    stop=(dt == DT - 1),
                                )
                            else:
                                # v tile [s, dv] = x_chunk.T @ wv_slice
                                nc.tensor.matmul(
                                    ps,
                                    lhsT=xc[:, dt, t * P:(t + 1) * P],
                                    rhs=w_sb[:, dt, :],
                                    start=(dt == 0),
                                    stop=(dt == DT - 1),
                                )
                        stg = stage.tile(
                            [P, 512], f32r, tag="stage", name="stg_proj"
                        )
                        nc.vector.tensor_copy(stg, ps)
                        if transposed:
                            nc.sync.dma_start(
                                outd[t * P:(t + 1) * P, sc * SC:(sc + 1) * SC],
                                stg,
                            )
                        else:
                            st_glob = sc * 4 + t
                            nc.sync.dma_start(
                                outd[st_glob * P:(st_glob + 1) * P, :], stg
                            )

        # ---------- Phase 2/3 pools ----------
        opool = ctx.enter_context(tc.tile_pool(name="opool", bufs=1))
        attp = ctx.enter_context(tc.tile_pool(name="attp", bufs=1))
        apool = ctx.enter_context(tc.tile_pool(name="apool", bufs=2))
        ppool = ctx.enter_context(tc.tile_pool(name="ppool", bufs=4))
        rpool = ctx.enter_context(tc.tile_pool(name="rpool", bufs=2))

        # ---------- Phase 3 weights (prefetch during phase 2) ----------
        woTs = opool.tile([P, HPC, D], f32r, tag="wo", name="woTs")
        nc.sync.dma_start(woTs, woT.rearrange("(hh p) m -> p hh m", p=P))

        # ---------- Phase 2: causal attention per head ----------
        vd_r = vd.rearrange("(st p) j -> p st j", p=P)
        attTs = []
        for h in range(HPC):
            kT_sb = apool.tile([P, S], f32r, tag="kT", name=f"kT_{h}")
            nc.sync.dma_start(kT_sb, kTd[h * P:(h + 1) * P, :])
            qT_sb = apool.tile([P, S], f32r, tag="qT", name=f"qT_{h}")
            nc.sync.dma_start(qT_sb, qTd[h * P:(h + 1) * P, :])
            v_sb = apool.tile([P, NST, HEAD_DIM], f32r, tag="v", name=f"v_{h}")
            nc.sync.dma_start(
                v_sb, vd_r[:, :, h * HEAD_DIM:(h + 1) * HEAD_DIM]
            )
            attT = attp.tile([P, S], f32r, tag=f"attT{h}", name=f"attT_{h}")
            attTs.append(attT)

            for ic in range(NSC):
                njt = 4 * ic + 4  # causal: j-tiles 0..njt-1 for this i-chunk
                ps_den = psum.tile([P, SC], f32, tag="den", name="ps_den")
                ps_pv = psum.tile([P, SC], f32, tag="pv", name="ps_pv")
                for jt in range(njt):
                    ps_s = psum.tile([P, SC], f32, tag="score", name="ps_s")
                    # scoresT[j, i] = k_h @ q_h.T (scale fused into exp)
                    nc.tensor.matmul(
                        ps_s,
                        lhsT=kT_sb[:, jt * P:(jt + 1) * P],
                        rhs=qT_sb[:, ic * SC:(ic + 1) * SC],
                        start=True,
                        stop=True,
                    )
                    pt = ppool.tile([P, SC], f32r, tag="prob", name="pt")
                    t = jt - 4 * ic
                    nc.scalar.activation(pt, ps_s, FT.Exp, scale=SCALE)
                    if t >= 0:
                        # diagonal-chunk tile: apply precomputed causal mask
                        nc.vector.tensor_mul(
                            out=pt,
                            in0=pt,
                            in1=mask_sb[:, t * SC:(t + 1) * SC],
                        )
                    # denominator: every partition row gets sum_j probsT[j,i]
                    nc.tensor.matmul(
                        ps_den,
                        lhsT=ones_sb,
                        rhs=pt,
                        start=(jt == 0),
                        stop=(jt == njt - 1),
                        skip_group_check=True,
                    )
                    # attT[dv, i] += v_h[j, dv].T-as-stationary @ probsT[j, i]
                    nc.tensor.matmul(
                        ps_pv,
                        lhsT=v_sb[:, jt, :],
                        rhs=pt,
                        start=(jt == 0),
                        stop=(jt == njt - 1),
                        skip_group_check=True,
                    )
                rec = rpool.tile([P, SC], f32, tag="rec", name="rec")
                nc.vector.reciprocal(rec, ps_den)
                nc.vector.tensor_mul(
                    out=attT[:, ic * SC:(ic + 1) * SC], in0=ps_pv, in1=rec
                )

        # ---------- Phase 3: partial o-projection ----------
        for st in range(NST):
            for mc in range(D // SC):
                ps = psum.tile([P, SC], f32, tag="pj", name="ps_o")
                for hh in range(HPC):
                    nc.tensor.matmul(
                        ps,
                        lhsT=attTs[hh][:, st * P:(st + 1) * P],
                        rhs=woTs[:, hh, mc * SC:(mc + 1) * SC],
                        start=(hh == 0),
                        stop=(hh == HPC - 1),
                    )
                og = stage.tile([P, SC], f32, tag="ostage", name="og")
                nc.vector.tensor_copy(og, ps)
                nc.sync.dma_start(
                    out[st * P:(st + 1) * P, mc * SC:(mc + 1) * SC], og
                )

    nc.compile()
    _NC_CACHE = nc
    return nc


def make_in_maps(x, wq, wk, wv, wo):
    x = np.asarray(x, dtype=np.float32)
    wq = np.asarray(wq, dtype=np.float32)
    wk = np.asarray(wk, dtype=np.float32)
    wv = np.asarray(wv, dtype=np.float32)
    wo = np.asarray(wo, dtype=np.float32)
    # mask[j, t*SC + i] = 1 where key j*... <= query i for the diagonal-chunk
    # tile at block-offset t: keep iff j_local <= i_local - t*128
    jj = np.arange(P)[:, None]
    ii = np.arange(SC)[None, :]
    causal = np.concatenate(
        [(jj <= ii - t * P).astype(np.float32) for t in range(4)], axis=1
    )
    ones = np.ones((P, P), dtype=np.float32)
    in_maps = []
    for c in range(N_CORES):
        b, g = divmod(c, HPC)
        j0 = g * JC
        in_maps.append(
            {
                "xT": np.ascontiguousarray(x[b].T),
                "wqT": np.ascontiguousarray(wq[j0:j0 + JC].T),
                "wkT": np.ascontiguousarray(wk[j0:j0 + JC].T),
                "wvT": np.ascontiguousarray(wv[j0:j0 + JC].T),
                "woT": np.ascontiguousarray(wo[:, j0:j0 + JC].T),
                "mask": causal,
                "ones": ones,
            }
        )
    return in_maps


def combine_outputs(results):
    out = np.zeros((B, S, D), dtype=np.float32)
    for c in range(N_CORES):
        out[c // HPC] += results[c]["out"]
    return out


def kernel(x, wq, wk, wv, wo):
    from concourse.bass_utils import run_bass_kernel_spmd

    nc = build_module()
    in_maps = make_in_maps(x, wq, wk, wv, wo)
    res = run_bass_kernel_spmd(nc, in_maps, list(range(N_CORES)))
    return combine_outputs(res.results)


# revision 10
# speedup vs baseline: 14.0185x; 14.0185x over previous
"""Multi-head causal attention (B=2, S=2048, D=2048, 16 heads) on 8 TRN2 cores.

Sharding: 2-way batch parallel x 4-way head tensor-parallel (4 heads/core).
Each core computes q/k/v projections for its 4 heads, causal softmax
attention, and a partial o-projection; the host sums the 4 partials per batch.

Host pre-transposes x and the weight slices so every on-chip matmul has its
contraction dim on SBUF partitions (no on-chip transposes at all):
  xT  [D, S]   = x[b].T
  wqT [D, JC]  = wq[j0:j0+512, :].T     (same wkT, wvT)
  woT [JC, D]  = wo[:, j0:j0+512].T

On-chip dataflow (per core), all matmuls in float32r (FP22 multiply,
fp32 accumulate in PSUM):
  phase 1: qT[j,s], kT[j,s] (lhsT=wT tile, rhs=xT tile) and v[s,dv]
           (lhsT=xT tile, rhs=wvT tile), spilled to DRAM scratch.
  phase 2 (per head): scoresT[j,i] = k_h @ q_h.T; exp on ScalarE (scale
           fused); causal handled by skipping j>i tiles + masking diagonal
           tiles; denominator via ones[128,128] @ probsT (gives the column
           sums broadcast across all partitions); attT[dv,i] accumulated
           with v tiles stationary; normalized by DVE reciprocal+mul.
  phase 3: out[s,m] partial = sum_h attT_h.T @ woT_h, DMA'd to DRAM.
"""

import math

import numpy as np

B, S, D = 2, 2048, 2048
HEADS, HEAD_DIM = 16, 128
P = 128
JC = 512          # per-core projection width (4 heads x 128)
SC = 512          # s-chunk / matmul moving width
DT = D // P       # 16 contraction tiles
NSC = S // SC     # 4 s-chunks
NST = S // P      # 16 s-tiles
HPC = 4           # heads per core
N_CORES = 8
SCALE = 1.0 / math.sqrt(HEAD_DIM)

_NC_CACHE = {}


def build_module(reps=1):
    """Build + compile the (single-program SPMD) Bass module once.

    reps>1 repeats the whole kernel body inside one NEFF (for timing:
    differencing per-call wall times cancels the fixed dispatch overhead).
    """
    if reps in _NC_CACHE:
        return _NC_CACHE[reps]

    from contextlib import ExitStack

    import concourse.tile as tile
    from concourse import bacc
    import concourse.mybir as mybir

    f32r = mybir.dt.float32r
    f32 = mybir.dt.float32
    FT = mybir.ActivationFunctionType

    nc = bacc.Bacc(
        "TRN2", target_bir_lowering=False, debug=False, num_devices=N_CORES
    )

    xT = nc.dram_tensor("xT", [D, S], f32r, kind="ExternalInput").ap()
    wqT = nc.dram_tensor("wqT", [D, JC], f32r, kind="ExternalInput").ap()
    wkT = nc.dram_tensor("wkT", [D, JC], f32r, kind="ExternalInput").ap()
    wvT = nc.dram_tensor("wvT", [D, JC], f32r, kind="ExternalInput").ap()
    woT = nc.dram_tensor("woT", [JC, D], f32r, kind="ExternalInput").ap()
    # mask[p, t*SC + i] : causal mask for the diagonal-chunk tile at offset t
    mask = nc.dram_tensor("mask", [P, 4 * SC], f32r, kind="ExternalInput").ap()
    ones = nc.dram_tensor("ones", [P, P], f32r, kind="ExternalInput").ap()
    out = nc.dram_tensor("out", [S, D], f32, kind="ExternalOutput").ap()

    qTd = nc.dram_tensor("qTd", [JC, S], f32r, kind="Internal").ap()
    kTd = nc.dram_tensor("kTd", [JC, S], f32r, kind="Internal").ap()
    vd = nc.dram_tensor("vd", [S, JC], f32r, kind="Internal").ap()

    with tile.TileContext(nc) as tc, ExitStack() as ctx:
        consts = ctx.enter_context(tc.tile_pool(name="consts", bufs=1))
        stage = ctx.enter_context(tc.tile_pool(name="stage", bufs=4))
        psum = ctx.enter_context(tc.tile_pool(name="psum", bufs=2, space="PSUM"))

        mask_sb = consts.tile([P, 4 * SC], f32r, tag="mask", name="mask_sb")
        nc.sync.dma_start(mask_sb, mask)
        ones_sb = consts.tile([P, P], f32r, tag="ones", name="ones_sb")
        nc.sync.dma_start(ones_sb, ones)

        xT_r = xT.rearrange("(dt p) s -> p dt s", p=P)
        vd_r = vd.rearrange("(st p) j -> p st j", p=P)

        for _rep in range(reps):
            # ---------- Phase 1: q/k/v projections, spilled to DRAM ----------
            # Scoped pools: weights (96KB/part) + x chunks (64KB/part) release
            # before the attention pools are created (SBUF is a stack
            # allocator).
            with ExitStack() as p1:
                wpool = p1.enter_context(tc.tile_pool(name="wpool", bufs=1))
                xpool = p1.enter_context(tc.tile_pool(name="xpool", bufs=2))

                wq_sb = wpool.tile([P, DT, JC], f32r, tag="wq", name="wq_sb")
                nc.sync.dma_start(
                    wq_sb, wqT.rearrange("(dt p) j -> p dt j", p=P)
                )
                wk_sb = wpool.tile([P, DT, JC], f32r, tag="wk", name="wk_sb")
                nc.sync.dma_start(
                    wk_sb, wkT.rearrange("(dt p) j -> p dt j", p=P)
                )
                wv_sb = wpool.tile([P, DT, JC], f32r, tag="wv", name="wv_sb")
                nc.sync.dma_start(
                    wv_sb, wvT.rearrange("(dt p) j -> p dt j", p=P)
                )

                for sc in range(NSC):
                    xc = xpool.tile([P, DT, SC], f32r, tag="x", name=f"xc_{sc}")
                    nc.sync.dma_start(xc, xT_r[:, :, sc * SC:(sc + 1) * SC])
                    for w_sb, outd, transposed in (
                        (wq_sb, qTd, True),
                        (wk_sb, kTd, True),
                        (wv_sb, vd, False),
                    ):
                        for t in range(4):
                            ps = psum.tile(
                                [P, 512], f32, tag="pj", name="ps_proj"
                            )
                            for dt in range(DT):
                                if transposed:
                                    # qT/kT tile [j, s] = w_slice.T @ x_chunk
                                    nc.tensor.matmul(
                                        ps,
                                        lhsT=w_sb[:, dt, t * P:(t + 1) * P],
                                        rhs=xc[:, dt, :],
                                        start=(dt == 0),
                                        stop=(dt == DT - 1),
                                    )
                                else:
                                    # v tile [s, dv] = x_chunk.T @ wv_slice
                                    nc.tensor.matmul(
                                        ps,
                                        lhsT=xc[:, dt, t * P:(t + 1) * P],
                                        rhs=w_sb[:, dt, :],
                                        start=(dt == 0),
                                        stop=(dt == DT - 1),
                                    )
                            stg = stage.tile(
                                [P, 512], f32r, tag="stage", name="stg_proj"
                            )
                            nc.vector.tensor_copy(stg, ps)
                            if transposed:
                                nc.sync.dma_start(
                                    outd[
                                        t * P:(t + 1) * P,
                                        sc * SC:(sc + 1) * SC,
                                    ],
                                    stg,
                                )
                            else:
                                st_glob = sc * 4 + t
                                nc.sync.dma_start(
                                    outd[st_glob * P:(st_glob + 1) * P, :], stg
                                )

            # ---------- Phase 2/3 pools ----------
            with ExitStack() as p2:
                opool = p2.enter_context(tc.tile_pool(name="opool", bufs=1))
                attp = p2.enter_context(tc.tile_pool(name="attp", bufs=1))
                apool = p2.enter_context(tc.tile_pool(name="apool", bufs=2))
                ppool = p2.enter_context(tc.tile_pool(name="ppool", bufs=4))
                rpool = p2.enter_context(tc.tile_pool(name="rpool", bufs=2))

                # Phase 3 weights (prefetch during phase 2)
                woTs = opool.tile([P, HPC, D], f32r, tag="wo", name="woTs")
                nc.sync.dma_start(woTs, woT.rearrange("(hh p) m -> p hh m", p=P))

                # ---------- Phase 2: causal attention per head ----------
                attTs = []
                for h in range(HPC):
                    kT_sb = apool.tile([P, S], f32r, tag="kT", name=f"kT_{h}")
                    nc.sync.dma_start(kT_sb, kTd[h * P:(h + 1) * P, :])
                    qT_sb = apool.tile([P, S], f32r, tag="qT", name=f"qT_{h}")
                    nc.sync.dma_start(qT_sb, qTd[h * P:(h + 1) * P, :])
                    v_sb = apool.tile(
                        [P, NST, HEAD_DIM], f32r, tag="v", name=f"v_{h}"
                    )
                    nc.sync.dma_start(
                        v_sb, vd_r[:, :, h * HEAD_DIM:(h + 1) * HEAD_DIM]
                    )
                    attT = attp.tile([P, S], f32r, tag=f"attT{h}", name=f"attT_{h}")
                    attTs.append(attT)

                    for ic in range(NSC):
                        njt = 4 * ic + 4  # causal: j-tiles 0..njt-1
                        ps_den = psum.tile([P, SC], f32, tag="den", name="ps_den")
                        ps_pv = psum.tile([P, SC], f32, tag="pv", name="ps_pv")
                        for jt in range(njt):
                            ps_s = psum.tile([P, SC], f32, tag="score", name="ps_s")
                            # scoresT[j, i] = k_h @ q_h.T (scale fused in exp)
                            nc.tensor.matmul(
                                ps_s,
                                lhsT=kT_sb[:, jt * P:(jt + 1) * P],
                                rhs=qT_sb[:, ic * SC:(ic + 1) * SC],
                                start=True,
                                stop=True,
                            )
                            pt = ppool.tile([P, SC], f32r, tag="prob", name="pt")
                            t = jt - 4 * ic
                            nc.scalar.activation(pt, ps_s, FT.Exp, scale=SCALE)
                            if t >= 0:
                                # diagonal-chunk tile: precomputed causal mask
                                nc.vector.tensor_mul(
                                    out=pt,
                                    in0=pt,
                                    in1=mask_sb[:, t * SC:(t + 1) * SC],
                                )
                            # denominator: every partition row gets
                            # sum_j probsT[j, i]
                            nc.tensor.matmul(
                                ps_den,
                                lhsT=ones_sb,
                                rhs=pt,
                                start=(jt == 0),
                                stop=(jt == njt - 1),
                                skip_group_check=True,
                            )
                            # attT[dv, i] += v_h[j, dv] stationary @ probsT
                            nc.tensor.matmul(
                                ps_pv,
                                lhsT=v_sb[:, jt, :],
                                rhs=pt,
                                start=(jt == 0),
                                stop=(jt == njt - 1),
                                skip_group_check=True,
                            )
                        rec = rpool.tile([P, SC], f32, tag="rec", name="rec")
                        nc.vector.reciprocal(rec, ps_den)
                        nc.vector.tensor_mul(
                            out=attT[:, ic * SC:(ic + 1) * SC],
                            in0=ps_pv,
                            in1=rec,
                        )

                # ---------- Phase 3: partial o-projection ----------
                for st in range(NST):
                    for mc in range(D // SC):
                        ps = psum.tile([P, SC], f32, tag="pj", name="ps_o")
                        for hh in range(HPC):
                            nc.tensor.matmul(
                                ps,
                                lhsT=attTs[hh][:, st * P:(st + 1) * P],
                                rhs=woTs[:, hh, mc * SC:(mc + 1) * SC],
                                start=(hh == 0),
                                stop=(hh == HPC - 1),
                            )
                        og = stage.tile([P, SC], f32, tag="ostage", name="og")
                        nc.vector.tensor_copy(og, ps)
                        nc.sync.dma_start(
                            out[st * P:(st + 1) * P, mc * SC:(mc + 1) * SC], og
                        )

    nc.compile()
    _NC_CACHE[reps] = nc
    return nc


def make_in_maps(x, wq, wk, wv, wo):
    x = np.asarray(x, dtype=np.float32)
    wq = np.asarray(wq, dtype=np.float32)
    wk = np.asarray(wk, dtype=np.float32)
    wv = np.asarray(wv, dtype=np.float32)
    wo = np.asarray(wo, dtype=np.float32)
    # mask[j, t*SC + i] = 1 where the key is visible to the query for the
    # diagonal-chunk tile at block-offset t: keep iff j_local <= i_local - t*128
    jj = np.arange(P)[:, None]
    ii = np.arange(SC)[None, :]
    causal = np.concatenate(
        [(jj <= ii - t * P).astype(np.float32) for t in range(4)], axis=1
    )
    ones = np.ones((P, P), dtype=np.float32)
    in_maps = []
    for c in range(N_CORES):
        b, g = divmod(c, HPC)
        j0 = g * JC
        in_maps.append(
            {
                "xT": np.ascontiguousarray(x[b].T),
                "wqT": np.ascontiguousarray(wq[j0:j0 + JC].T),
                "wkT": np.ascontiguousarray(wk[j0:j0 + JC].T),
                "wvT": np.ascontiguousarray(wv[j0:j0 + JC].T),
                "woT": np.ascontiguousarray(wo[:, j0:j0 + JC].T),
                "mask": causal,
                "ones": ones,
            }
        )
    return in_maps


def combine_outputs(results):
    out = np.zeros((B, S, D), dtype=np.float32)
    for c in range(N_CORES):
        out[c // HPC] += results[c]["out"]
    return out


def kernel(x, wq, wk, wv, wo):
    from concourse.bass_utils import run_bass_kernel_spmd

    nc = build_module()
    in_maps = make_in_maps(x, wq, wk, wv, wo)
    res = run_bass_kernel_spmd(nc, in_maps, list(range(N_CORES)))
    return combine_outputs(res.results)


# revision 12
# speedup vs baseline: 14.8742x; 1.0610x over previous
"""Multi-head causal attention (B=2, S=2048, D=2048, 16 heads) on 8 TRN2 cores.

Sharding: 2-way batch parallel x 4-way head tensor-parallel (4 heads/core).
Each core computes q/k/v projections for its 4 heads, causal softmax
attention, and a partial o-projection; the host sums the 4 partials per batch.

Host pre-transposes x and the weight slices so every on-chip matmul has its
contraction dim on SBUF partitions (no on-chip transposes at all):
  xT  [D, S]   = x[b].T
  wqT [D, JC]  = wq[j0:j0+512, :].T     (same wkT, wvT)
  woT [JC, D]  = wo[:, j0:j0+512].T

On-chip dataflow (per core), all matmuls in float32r (FP22 multiply,
fp32 accumulate in PSUM):
  phase 1: qT[j,s], kT[j,s] (lhsT=wT tile, rhs=xT tile) and v[s,dv]
           (lhsT=xT tile, rhs=wvT tile), spilled to DRAM scratch.
  phase 2 (per head, per 512-wide i-chunk):
           a) scoresT[j,i] = k_h @ q_h.T per j-tile, exp on ScalarE (scale
              fused), diagonal j-tiles trimmed to i >= j and masked with a
              triangular 128x128 mask;
           b) denominator: ones[128,128] @ probsT accumulated over j-tiles
              (every PSUM partition gets the column sum - broadcast built in);
           c) attT[dv,i] += v_h[j,dv]-stationary @ probsT[j,i] over j-tiles;
           normalize with DVE reciprocal + multiply.
  phase 3: out[s,m] partial = sum_h attT_h.T @ woT_h, DMA'd to DRAM.
"""

import math

import numpy as np

B, S, D = 2, 2048, 2048
HEADS, HEAD_DIM = 16, 128
P = 128
JC = 512          # per-core projection width (4 heads x 128)
SC = 512          # s-chunk / matmul moving width
DT = D // P       # 16 contraction tiles
NSC = S // SC     # 4 s-chunks
NST = S // P      # 16 s-tiles
HPC = 4           # heads per core
N_CORES = 8
SCALE = 1.0 / math.sqrt(HEAD_DIM)

_NC_CACHE = {}


def build_module(reps=1):
    """Build + compile the (single-program SPMD) Bass module once.

    reps>1 repeats the whole kernel body inside one NEFF (for timing:
    differencing per-call wall times cancels the fixed dispatch overhead).
    """
    if reps in _NC_CACHE:
        return _NC_CACHE[reps]

    from contextlib import ExitStack

    import concourse.tile as tile
    from concourse import bacc
    import concourse.mybir as mybir

    f32r = mybir.dt.float32r
    f32 = mybir.dt.float32
    FT = mybir.ActivationFunctionType

    nc = bacc.Bacc(
        "TRN2", target_bir_lowering=False, debug=False, num_devices=N_CORES
    )

    xT = nc.dram_tensor("xT", [D, S], f32r, kind="ExternalInput").ap()
    wqT = nc.dram_tensor("wqT", [D, JC], f32r, kind="ExternalInput").ap()
    wkT = nc.dram_tensor("wkT", [D, JC], f32r, kind="ExternalInput").ap()
    wvT = nc.dram_tensor("wvT", [D, JC], f32r, kind="ExternalInput").ap()
    woT = nc.dram_tensor("woT", [JC, D], f32r, kind="ExternalInput").ap()
    # mask[j, c] = 1 iff j <= c : causal triangle for a diagonal 128-block
    mask = nc.dram_tensor("mask", [P, P], f32r, kind="ExternalInput").ap()
    ones = nc.dram_tensor("ones", [P, P], f32r, kind="ExternalInput").ap()
    out = nc.dram_tensor("out", [S, D], f32, kind="ExternalOutput").ap()

    qTd = nc.dram_tensor("qTd", [JC, S], f32r, kind="Internal").ap()
    kTd = nc.dram_tensor("kTd", [JC, S], f32r, kind="Internal").ap()
    vd = nc.dram_tensor("vd", [S, JC], f32r, kind="Internal").ap()

    with tile.TileContext(nc) as tc, ExitStack() as ctx:
        consts = ctx.enter_context(tc.tile_pool(name="consts", bufs=1))
        stage = ctx.enter_context(tc.tile_pool(name="stage", bufs=4))

        mask_sb = consts.tile([P, P], f32r, tag="mask", name="mask_sb")
        nc.sync.dma_start(mask_sb, mask)
        ones_sb = consts.tile([P, P], f32r, tag="ones", name="ones_sb")
        nc.sync.dma_start(ones_sb, ones)

        xT_r = xT.rearrange("(dt p) s -> p dt s", p=P)
        vd_r = vd.rearrange("(st p) j -> p st j", p=P)

        for _rep in range(reps):
            # ---------- Phase 1: q/k/v projections, spilled to DRAM ----------
            # Scoped pools: weights (96KB/part) + x chunks (64KB/part) release
            # before the attention pools are created (SBUF is a stack
            # allocator).
            with ExitStack() as p1:
                wpool = p1.enter_context(tc.tile_pool(name="wpool", bufs=1))
                xpool = p1.enter_context(tc.tile_pool(name="xpool", bufs=2))
                psum1 = p1.enter_context(
                    tc.tile_pool(name="psum1", bufs=2, space="PSUM")
                )

                wq_sb = wpool.tile([P, DT, JC], f32r, tag="wq", name="wq_sb")
                nc.sync.dma_start(
                    wq_sb, wqT.rearrange("(dt p) j -> p dt j", p=P)
                )
                wk_sb = wpool.tile([P, DT, JC], f32r, tag="wk", name="wk_sb")
                nc.sync.dma_start(
                    wk_sb, wkT.rearrange("(dt p) j -> p dt j", p=P)
                )
                wv_sb = wpool.tile([P, DT, JC], f32r, tag="wv", name="wv_sb")
                nc.sync.dma_start(
                    wv_sb, wvT.rearrange("(dt p) j -> p dt j", p=P)
                )

                for sc in range(NSC):
                    xc = xpool.tile([P, DT, SC], f32r, tag="x", name=f"xc_{sc}")
                    nc.sync.dma_start(xc, xT_r[:, :, sc * SC:(sc + 1) * SC])
                    for w_sb, outd, transposed in (
                        (wq_sb, qTd, True),
                        (wk_sb, kTd, True),
                        (wv_sb, vd, False),
                    ):
                        for t in range(4):
                            ps = psum1.tile(
                                [P, 512], f32, tag="pj", name="ps_proj"
                            )
                            for dt in range(DT):
                                if transposed:
                                    # qT/kT tile [j, s] = w_slice.T @ x_chunk
                                    nc.tensor.matmul(
                                        ps,
                                        lhsT=w_sb[:, dt, t * P:(t + 1) * P],
                                        rhs=xc[:, dt, :],
                                        start=(dt == 0),
                                        stop=(dt == DT - 1),
                                    )
                                else:
                                    # v tile [s, dv] = x_chunk.T @ wv_slice
                                    nc.tensor.matmul(
                                        ps,
                                        lhsT=xc[:, dt, t * P:(t + 1) * P],
                                        rhs=w_sb[:, dt, :],
                                        start=(dt == 0),
                                        stop=(dt == DT - 1),
                                    )
                            stg = stage.tile(
                                [P, 512], f32r, tag="stage", name="stg_proj"
                            )
                            nc.vector.tensor_copy(stg, ps)
                            if transposed:
                                nc.sync.dma_start(
                                    outd[
                                        t * P:(t + 1) * P,
                                        sc * SC:(sc + 1) * SC,
                                    ],
                                    stg,
                                )
                            else:
                                st_glob = sc * 4 + t
                                nc.sync.dma_start(
                                    outd[st_glob * P:(st_glob + 1) * P, :], stg
                                )

            # ---------- Phase 2/3 pools ----------
            with ExitStack() as p2:
                opool = p2.enter_context(tc.tile_pool(name="opool", bufs=1))
                attp = p2.enter_context(tc.tile_pool(name="attp", bufs=1))
                apool = p2.enter_context(tc.tile_pool(name="apool", bufs=2))
                ppool = p2.enter_context(tc.tile_pool(name="ppool", bufs=17))
                rpool = p2.enter_context(tc.tile_pool(name="rpool", bufs=2))
                psum2 = p2.enter_context(
                    tc.tile_pool(name="psum2", bufs=2, space="PSUM")
                )

                # Phase 3 weights (prefetch during phase 2)
                woTs = opool.tile([P, HPC, D], f32r, tag="wo", name="woTs")
                nc.sync.dma_start(woTs, woT.rearrange("(hh p) m -> p hh m", p=P))

                # ---------- Phase 2: causal attention per head ----------
                attTs = []
                for h in range(HPC):
                    kT_sb = apool.tile([P, S], f32r, tag="kT", name=f"kT_{h}")
                    nc.sync.dma_start(kT_sb, kTd[h * P:(h + 1) * P, :])
                    qT_sb = apool.tile([P, S], f32r, tag="qT", name=f"qT_{h}")
                    nc.sync.dma_start(qT_sb, qTd[h * P:(h + 1) * P, :])
                    v_sb = apool.tile(
                        [P, NST, HEAD_DIM], f32r, tag="v", name=f"v_{h}"
                    )
                    nc.sync.dma_start(
                        v_sb, vd_r[:, :, h * HEAD_DIM:(h + 1) * HEAD_DIM]
                    )
                    attT = attp.tile(
                        [P, S], f32r, tag=f"attT{h}", name=f"attT_{h}"
                    )
                    attTs.append(attT)

                    for ic in range(NSC):
                        njt = 4 * ic + 4  # causal: j-tiles 0..njt-1
                        # offs[jt]: first useful i_local column of tile jt
                        offs = [
                            max(0, (jt - 4 * ic) * P) for jt in range(njt)
                        ]

                        # a) scoresT + exp (+ diagonal mask)
                        pts = []
                        for jt in range(njt):
                            off = offs[jt]
                            ps_s = psum2.tile(
                                [P, SC], f32, tag="score", bufs=3, name="ps_s"
                            )
                            nc.tensor.matmul(
                                ps_s[:, off:],
                                lhsT=kT_sb[:, jt * P:(jt + 1) * P],
                                rhs=qT_sb[:, ic * SC + off:(ic + 1) * SC],
                                start=True,
                                stop=True,
                            )
                            pt = ppool.tile([P, SC], f32r, tag="prob", name="pt")
                            pts.append(pt)
                            nc.scalar.activation(
                                pt[:, off:], ps_s[:, off:], FT.Exp, scale=SCALE
                            )
                            if jt >= 4 * ic:
                                # triangular mask on the diagonal 128-block
                                nc.vector.tensor_mul(
                                    out=pt[:, off:off + P],
                                    in0=pt[:, off:off + P],
                                    in1=mask_sb,
                                )

                        # b) denominator (ones stationary loaded once)
                        ps_den = psum2.tile(
                            [P, SC], f32, tag="den", name="ps_den"
                        )
                        for jt in range(njt):
                            off = offs[jt]
                            nc.tensor.matmul(
                                ps_den[:, off:],
                                lhsT=ones_sb,
                                rhs=pts[jt][:, off:],
                                start=(jt == 0),
                                stop=(jt == njt - 1),
                                skip_group_check=True,
                            )
                        rec = rpool.tile([P, SC], f32, tag="rec", name="rec")
                        nc.vector.reciprocal(rec, ps_den)

                        # c) attT[dv, i] += v_h[j, dv] stationary @ probsT
                        ps_pv = psum2.tile([P, SC], f32, tag="pv", name="ps_pv")
                        for jt in range(njt):
                            off = offs[jt]
                            nc.tensor.matmul(
                                ps_pv[:, off:],
                                lhsT=v_sb[:, jt, :],
                                rhs=pts[jt][:, off:],
                                start=(jt == 0),
                                stop=(jt == njt - 1),
                                skip_group_check=True,
                            )
                        nc.vector.tensor_mul(
                            out=attT[:, ic * SC:(ic + 1) * SC],
                            in0=ps_pv,
                            in1=rec,
                        )

                # ---------- Phase 3: partial o-projection ----------
                for st in range(NST):
                    for mc in range(D // SC):
                        ps = psum2.tile(
                            [P, SC], f32, tag="score", bufs=3, name="ps_o"
                        )
                        for hh in range(HPC):
                            nc.tensor.matmul(
                                ps,
                                lhsT=attTs[hh][:, st * P:(st + 1) * P],
                                rhs=woTs[:, hh, mc * SC:(mc + 1) * SC],
                                start=(hh == 0),
                                stop=(hh == HPC - 1),
                            )
                        og = stage.tile([P, SC], f32, tag="ostage", name="og")
                        nc.vector.tensor_copy(og, ps)
                        nc.sync.dma_start(
                            out[st * P:(st + 1) * P, mc * SC:(mc + 1) * SC], og
                        )

    nc.compile()
    _NC_CACHE[reps] = nc
    return nc


def make_in_maps(x, wq, wk, wv, wo):
    x = np.asarray(x, dtype=np.float32)
    wq = np.asarray(wq, dtype=np.float32)
    wk = np.asarray(wk, dtype=np.float32)
    wv = np.asarray(wv, dtype=np.float32)
    wo = np.asarray(wo, dtype=np.float32)
    # mask[j, c] = 1 iff key j visible to query c within a diagonal block
    causal = np.triu(np.ones((P, P), dtype=np.float32))
    ones = np.ones((P, P), dtype=np.float32)
    in_maps = []
    for c in range(N_CORES):
        b, g = divmod(c, HPC)
        j0 = g * JC
        in_maps.append(
            {
                "xT": np.ascontiguousarray(x[b].T),
                "wqT": np.ascontiguousarray(wq[j0:j0 + JC].T),
                "wkT": np.ascontiguousarray(wk[j0:j0 + JC].T),
                "wvT": np.ascontiguousarray(wv[j0:j0 + JC].T),
                "woT": np.ascontiguousarray(wo[:, j0:j0 + JC].T),
                "mask": causal,
                "ones": ones,
            }
        )
    return in_maps


def combine_outputs(results):
    out = np.zeros((B, S, D), dtype=np.float32)
    for c in range(N_CORES):
        out[c // HPC] += results[c]["out"]
    return out


def kernel(x, wq, wk, wv, wo):
    from concourse.bass_utils import run_bass_kernel_spmd

    nc = build_module()
    in_maps = make_in_maps(x, wq, wk, wv, wo)
    res = run_bass_kernel_spmd(nc, in_maps, list(range(N_CORES)))
    return combine_outputs(res.results)
